# revision 32
# baseline (speedup 1.0000x reference)
"""Trainium2 Bass kernel for nn_CrossAttention (B=2, L=1024, S=2048, DIM=1024, H=16 heads).

Sharding: tensor-parallel over heads x data-parallel over batch.
Core c handles batch b = c//4 and head-group g = c%4 (4 heads = 256 of the
1024 hidden channels).  Each core computes, for its (b, g):

    QT = (Wq_g)^T x_q^T          [256, 1024]   (m on partitions)
    KT = (Wk_g)^T x_k^T          [256, 2048]
    V  = x_v Wv_g                [2048, 256]   (s on partitions)
    per head h (d=64):
        ST_h = KT_h^T' ...       S^T[s, l] = k_s . q_l   (s on partitions)
        P_h  = exp(SCALE * ST_h)            (unnormalized, s on partitions)
        [O^T_h ; sums_h] = [V_h | 1]^T @ P_h   (ones-column folds the softmax
                                                denominator into the matmul)
        XgT_h = O^T_h * (1/sums_h)          (broadcast via K=1 matmul)
    out_partial = XgT^T @ Wo_g   [1024, 1024]  (bf16)

Host gathers: out[b] = sum_g out_partial[4b+g] + bo.

Weights are pre-rearranged on the host so every DMA is contiguous.
Activations and weights stream in bf16; the attention core runs in
fp32r (FP22) with fp32 PSUM accumulation.
"""

import sys

if "/opt/trn_rl_repo" not in sys.path:
    sys.path.insert(0, "/opt/trn_rl_repo")

import numpy as np

B, L, S, C = 2, 1024, 2048, 1024
NH, D = 16, 64          # total heads, head dim
HPC = 4                 # heads per core
M = HPC * D             # 256 output channels per core
SCALE = D ** -0.5
P = 128                 # partitions
NCORES = 8
CK = C // P             # 8 c-tiles
NST = S // P            # 16 s-tiles
LCH = 512               # l-chunk
NLCH = L // LCH         # 2

_cache = {}


def _build():
    import concourse.tile as tile
    from concourse import mybir, bacc

    f32 = mybir.dt.float32
    f32r = mybir.dt.float32r
    bf16 = mybir.dt.bfloat16

    nc = bacc.Bacc("TRN2", target_bir_lowering=False, debug=False)

    # xq/xk pre-packed host-side as [p, ckpair, ck_in_pair, cols] so each
    # per-pair DMA reads 4-8KB contiguous per partition
    xqP = nc.dram_tensor("xqP", [P, CK // 2, 2, L], bf16, kind="ExternalInput")
    xkP = nc.dram_tensor("xkP", [P, CK // 2, 2, S], bf16, kind="ExternalInput")
    xvT = nc.dram_tensor("xvT", [C, S], bf16, kind="ExternalInput")
    # weights pre-rearranged host-side: [p, ck, m] / [p, kt, n]
    wq = nc.dram_tensor("wq", [P, CK, M], bf16, kind="ExternalInput")
    wk = nc.dram_tensor("wk", [P, CK, M], bf16, kind="ExternalInput")
    wv = nc.dram_tensor("wv", [P, CK, M], bf16, kind="ExternalInput")
    wo = nc.dram_tensor("wo", [P, M // P, C], bf16, kind="ExternalInput")
    outp = nc.dram_tensor("outp", [L, C], bf16, kind="ExternalOutput")

    with tile.TileContext(nc) as tc:
        from concourse import library_config
        with tc.tile_pool(name="singles", bufs=1) as singles, \
             tc.tile_pool(name="acts", bufs=6) as acts, \
             tc.tile_pool(name="pts", bufs=36) as pts, \
             tc.tile_pool(name="small", bufs=2) as small, \
             tc.tile_pool(name="obuf", bufs=3) as obuf:

            # ---- persistent SBUF ----
            wq_sb = singles.tile([P, CK, M], bf16, tag="wq")
            wk_sb = singles.tile([P, CK, M], bf16, tag="wk")
            wv_sb = singles.tile([P, CK, M], bf16, tag="wv")
            wo_sb = singles.tile([P, M // P, C], bf16, tag="wo")
            xv_sb = singles.tile([P, CK, S], bf16, tag="xv")
            # A-critical loads split across the sync HWDGE queue and the
            # gpsimd SWDGE queue (both ~210-240 GB/s; together ~HBM cap).
            # Neither touches the ACT queue, which carries exp.
            nc.sync.dma_start(wq_sb[:, 0:CK // 2, :], wq[:, 0:CK // 2, :])
            nc.sync.dma_start(wq_sb[:, CK // 2:, :], wq[:, CK // 2:, :])

            # per-chunk tiles so dependency tracking stays fine-grained
            ktt = [[singles.tile([P, 512], bf16, tag=f"kt{mt}{nch}",
                                 name=f"ktt{mt}{nch}")
                    for nch in range(4)] for mt in range(2)]
            qtt = [[singles.tile([P, 512], bf16, tag=f"qt{mt}{lh}",
                                 name=f"qtt{mt}{lh}")
                    for lh in range(2)] for mt in range(2)]
            # per-head lhsT [s, 128]: col 0 = ones (sums -> psum row 0),
            # cols 64..127 = V (O^T -> psum rows 64..127, 32-aligned for DVE),
            # cols 1..63 junk -> unread psum rows
            vt = [singles.tile([P, HPC, P], bf16, tag=f"v{st}", name=f"vt{st}")
                  for st in range(NST)]
            xgt_sb = singles.tile([P, 2, L], bf16, tag="xgt")
            for st in range(NST):
                nc.gpsimd.memset(vt[st][:, :, 0:1], 1.0)

            # =========== Phase A: QT + KT projections ===========
            with tc.tile_pool(name="ps_proj", bufs=8, space="PSUM") as psp:
                # --- QT projection ---
                qt_ps = [psp.tile([P, 512], f32, tag="pp", name=f"qtps{i}") for i in range(4)]
                for ck in range(CK):
                    xq_t = acts.tile([P, L], bf16, tag="act")
                    nc.sync.dma_start(xq_t[:], xqP[:, ck // 2, ck % 2, :])
                    for mt in range(2):
                        for lh in range(2):
                            nc.tensor.matmul(
                                qt_ps[mt * 2 + lh][:],
                                wq_sb[:, ck, mt * P:(mt + 1) * P],
                                xq_t[:, lh * 512:(lh + 1) * 512],
                                start=(ck == 0), stop=(ck == CK - 1))
                            if ck == CK - 1:
                                if lh == 0:
                                    nc.scalar.copy(qtt[mt][lh][:],
                                                   qt_ps[mt * 2 + lh][:])
                                else:
                                    nc.vector.tensor_copy(qtt[mt][lh][:],
                                                          qt_ps[mt * 2 + lh][:])

                # --- KT projection ---
                kt_ps = [psp.tile([P, 512], f32, tag="pp", name=f"ktps{i}") for i in range(8)]
                nc.sync.dma_start(wk_sb[:], wk[:, :, :])
                for ck in range(CK):
                    xk_t = acts.tile([P, S], bf16, tag="actk")
                    nc.sync.dma_start(xk_t[:], xkP[:, ck // 2, ck % 2, :])
                    for mt in range(2):
                        for nch in range(4):
                            nc.tensor.matmul(
                                kt_ps[mt * 4 + nch][:],
                                wk_sb[:, ck, mt * P:(mt + 1) * P],
                                xk_t[:, nch * 512:(nch + 1) * 512],
                                start=(ck == 0), stop=(ck == CK - 1))
                            if ck == CK - 1:
                                if nch % 2 == 0:
                                    nc.scalar.copy(ktt[mt][nch][:],
                                                   kt_ps[mt * 4 + nch][:])
                                else:
                                    nc.vector.tensor_copy(ktt[mt][nch][:],
                                                          kt_ps[mt * 4 + nch][:])
                # wv/wo on scalar; xv (half-row-major chunks) trails
                # the A-critical stream on sync
                nc.sync.dma_start(wv_sb[:], wv[:, :, :])
                nc.sync.dma_start(wo_sb[:], wo[:, :, :])
                for h in range(2):
                    for ck in range(CK):
                        nc.sync.dma_start(
                            xv_sb[:, ck, h * 1024:(h + 1) * 1024],
                            xvT[ck * P:(ck + 1) * P, h * 1024:(h + 1) * 1024])
                # gpsimd library (for partition_broadcast): the load blocks
                # the Pool queue ~10us, harmless here (no SWDGE traffic)
                nc.gpsimd.load_library(library_config.standard)

            # =========== Phases B-E: V-proj / attention pipeline ===========
            def st_step(lch, pair, st):
                """ST pair matmuls + exp; returns the PT tile."""
                nch, co = st // 4, (st % 4) * P
                st_ps = pst.tile([P, 2, LCH], f32, tag="st", name=f"stps_{lch}_{pair}_{st}")
                nc.tensor.matmul(
                    st_ps[:, 0, :], ktt[pair][nch][0:D, co:co + P],
                    qtt[pair][lch][0:D, :], start=True, stop=True)
                nc.tensor.matmul(
                    st_ps[:, 1, :], ktt[pair][nch][D:P, co:co + P],
                    qtt[pair][lch][D:P, :], start=True, stop=True,
                    tile_position=(64, 0))
                pt_t = pts.tile([P, 2, LCH], bf16, tag="pt", name=f"pt_{lch}_{pair}_{st}")
                nc.scalar.activation(pt_t[:], st_ps[:],
                                     mybir.ActivationFunctionType.Exp, scale=SCALE)
                return pt_t

            def o_step(o_ps, lch, pair, st, pt_t):
                for hh in range(2):
                    nc.tensor.matmul(
                        o_ps[hh][:], vt[st][:, pair * 2 + hh, :], pt_t[:, hh, :],
                        start=(st == 0), stop=(st == NST - 1))

            def norm_pair(lch, pair, o_ps):
                """Tensor-free softmax normalization: sums sit on partition 0
                (ones column at index 0); approx-recip straight from PSUM,
                gpsimd partition-broadcast, then DVE multiply."""
                lsl = slice(lch * LCH, (lch + 1) * LCH)
                for hh in range(2):
                    rc = small.tile([1, LCH], f32, tag="rc")
                    bc_sb = small.tile([D, LCH], f32, tag="bc")
                    with nc.allow_low_precision(reason="softmax denom approx recip"):
                        nc.vector.reciprocal_approx_fast(rc[0:1, :],
                                                         o_ps[hh][0:1, :])
                    nc.gpsimd.partition_broadcast(bc_sb[:, :], rc[0:1, :])
                    nc.vector.tensor_mul(
                        xgt_sb[hh * D:(hh + 1) * D, pair, lsl],
                        o_ps[hh][P - D:P, :], bc_sb[:])

            ob_tiles = {}
            ob_n = [0]

            def wo_copy_store(lt, nch, wo_ps):
                if lt not in ob_tiles:
                    ob_tiles[lt] = obuf.tile([P, C], bf16, tag="ob", name=f"ob_{lt}")
                ob_sb = ob_tiles[lt]
                # PSUM->SBUF cast on ACT (idle once the exp stream ends);
                # combined per-lt stores alternating sync/gpsimd queues
                nc.scalar.copy(ob_sb[:, nch * 512:(nch + 1) * 512], wo_ps[:])
                ob_n[0] += 1
                if nch == 1:
                    eng = nc.sync if lt % 2 == 0 else nc.gpsimd
                    eng.dma_start(
                        outp[lt * P:(lt + 1) * P, :], ob_tiles.pop(lt)[:])

            def wo_step(ps1, lt, nch):
                wo_ps = ps1.tile([P, 512], f32, tag="ps1", name=f"wops_{lt}_{nch}")
                for kt in range(2):
                    nc.tensor.matmul(
                        wo_ps[:], xgt_sb[:, kt, lt * P:(lt + 1) * P],
                        wo_sb[:, kt, nch * 512:(nch + 1) * 512],
                        start=(kt == 0), stop=(kt == 1))
                wo_copy_store(lt, nch, wo_ps)

            pt0 = {}   # (pair, st) -> PT tile for lch 0
            pt1 = {}
            # PSUM pool lifetimes overlap non-hierarchically; use the two
            # allocator sides as independent stacks:
            #   right: pst [B,C] -> pse [D,E];  left: psv [B] -> ps1 [C,D,E]
            pst = tc.alloc_tile_pool(name="ps_st", bufs=2, side="right", space="PSUM")
            psv = tc.alloc_tile_pool(name="ps_v", bufs=4, side="left", space="PSUM")

            # --- Phase B: V projection (4 quarters) interleaved with
            #     lch0's ST+exp steps; first 4 ST steps up front so exp
            #     starts as soon as KT lands (xv may still be in flight) ---
            step = 0
            for _ in range(4):
                pair, st = divmod(step, NST)
                pt0[(pair, st)] = st_step(0, pair, st)
                step += 1
            for q in range(4):
                v_ps = [psv.tile([P, M], f32, tag="vp", name=f"vps{q}_{i}")
                        for i in range(4)]
                for ck in range(CK):
                    for st4 in range(4):
                        nc.tensor.matmul(
                            v_ps[st4][:],
                            xv_sb[:, ck, q * 4 * P + st4 * P:
                                  q * 4 * P + (st4 + 1) * P],
                            wv_sb[:, ck, :],
                            start=(ck == 0), stop=(ck == CK - 1))
                    if step < 2 * NST:
                        pair, st = divmod(step, NST)
                        pt0[(pair, st)] = st_step(0, pair, st)
                        step += 1
                for st4 in range(4):
                    st = q * 4 + st4
                    nc.vector.tensor_copy(
                        vt[st][:, :, P - D:P],
                        v_ps[st4][:].rearrange("p (h d) -> p h d", h=HPC))

            psv.release()
            ps1 = tc.alloc_tile_pool(name="ps1", bufs=4, side="left", space="PSUM")

            # --- Phase C0: O(0,p0) + ST/exp(1,p0) ---
            o00 = [ps1.tile([P, LCH], f32, tag="ps1",
                            name=f"ops0_0_{i}") for i in range(2)]
            for st in range(NST):
                o_step(o00, 0, 0, st, pt0.pop((0, st)))
                pt1[(0, st)] = st_step(1, 0, st)
            norm_pair(0, 0, o00)

            # --- Phase C1: O(0,p1) + ST/exp(1,p1) + O(1,p0) ---
            o01 = [ps1.tile([P, LCH], f32, tag="ps1",
                            name=f"ops0_1_{i}") for i in range(2)]
            o10 = [ps1.tile([P, LCH], f32, tag="ps1",
                            name=f"ops1_0_{i}") for i in range(2)]
            for st in range(NST):
                o_step(o01, 0, 1, st, pt0.pop((1, st)))
                pt1[(1, st)] = st_step(1, 1, st)
                o_step(o10, 1, 0, st, pt1.pop((0, st)))
            norm_pair(0, 1, o01)
            norm_pair(1, 0, o10)

            pst.release()
            pse = tc.alloc_tile_pool(name="ps_e", bufs=4, side="right", space="PSUM")

            # --- Phase D: O(1,p1) + Wo(lch0) + early kt0 of Wo(lch1) ---
            wo_jobs0 = [(lt, nch) for lt in range(4) for nch in range(2)]
            eps = {}
            o11 = [ps1.tile([P, LCH], f32, tag="ps1",
                            name=f"ops1_1_{i}") for i in range(2)]
            for st in range(NST):
                o_step(o11, 1, 1, st, pt1.pop((1, st)))
                if st % 2 == 1 and wo_jobs0:
                    wo_step(ps1, *wo_jobs0.pop(0))
                if st in (5, 7, 9, 11):
                    # early kt=0 half of phase-E Wo (xgt lch1 pair0 ready)
                    lt = 4 + (st - 5) // 2
                    wo_ps = pse.tile([P, 512], f32, tag="pse",
                                     name=f"ewo_{lt}")
                    nc.tensor.matmul(
                        wo_ps[:], xgt_sb[:, 0, lt * P:(lt + 1) * P],
                        wo_sb[:, 0, 0:512], start=True, stop=False)
                    eps[lt] = wo_ps
            norm_pair(1, 1, o11)
            for lt, nch in wo_jobs0:
                wo_step(ps1, lt, nch)

            # --- Phase E: Wo(lch1) ---
            for lt in range(4, 8):
                # finish the early-started nch=0 job
                wo_ps = eps.pop(lt)
                nc.tensor.matmul(
                    wo_ps[:], xgt_sb[:, 1, lt * P:(lt + 1) * P],
                    wo_sb[:, 1, 0:512], start=False, stop=True)
                wo_copy_store(lt, 0, wo_ps)
                wo_step(ps1, lt, 1)

            pse.release()
            ps1.release()

    nc.compile()
    return nc


def _get_nc():
    if "nc" not in _cache:
        _cache["nc"] = _build()
    return _cache["nc"]


def _make_in_maps(inputs):
    import ml_dtypes

    bf16 = ml_dtypes.bfloat16
    query = np.asarray(inputs["query"], dtype=np.float32)
    key = np.asarray(inputs["key"], dtype=np.float32)
    value = np.asarray(inputs["value"], dtype=np.float32)
    Wq = np.asarray(inputs["Wq"], dtype=np.float32)
    Wk = np.asarray(inputs["Wk"], dtype=np.float32)
    Wv = np.asarray(inputs["Wv"], dtype=np.float32)
    Wo = np.asarray(inputs["Wo"], dtype=np.float32)

    def wsplit(W, g):
        # [C, M] slice -> [P, CK, M] so the DMA is fully contiguous
        return np.ascontiguousarray(
            W[:, g * M:(g + 1) * M].reshape(CK, P, M).transpose(1, 0, 2)
        ).astype(bf16)

    def wosplit(W, g):
        # [M, C] slice -> [P, M//P, C]
        return np.ascontiguousarray(
            W[g * M:(g + 1) * M, :].reshape(M // P, P, C).transpose(1, 0, 2)
        ).astype(bf16)

    def pack_pairs(xT):
        # [C, cols] -> [P, CK//2, 2, cols]
        cols = xT.shape[1]
        return np.ascontiguousarray(
            xT.reshape(CK // 2, 2, P, cols).transpose(2, 0, 1, 3)).astype(bf16)

    qP = [pack_pairs(query[b].T) for b in range(B)]
    kP = [pack_pairs(key[b].T) for b in range(B)]
    vT = [np.ascontiguousarray(value[b].T).astype(bf16) for b in range(B)]
    wq_s = [wsplit(Wq, g) for g in range(4)]
    wk_s = [wsplit(Wk, g) for g in range(4)]
    wv_s = [wsplit(Wv, g) for g in range(4)]
    wo_s = [wosplit(Wo, g) for g in range(4)]

    in_maps = []
    for core in range(NCORES):
        b, g = core // 4, core % 4
        in_maps.append({
            "xqP": qP[b], "xkP": kP[b], "xvT": vT[b],
            "wq": wq_s[g], "wk": wk_s[g], "wv": wv_s[g], "wo": wo_s[g],
        })
    return in_maps


def kernel(query, key, value, Wq, Wk, Wv, Wo, bo):
    from concourse.bass_utils import run_bass_kernel_spmd

    nc = _get_nc()
    bo = np.asarray(bo, dtype=np.float32)
    in_maps = _make_in_maps(dict(query=query, key=key, value=value,
                                 Wq=Wq, Wk=Wk, Wv=Wv, Wo=Wo))

    res = run_bass_kernel_spmd(nc, in_maps, core_ids=list(range(NCORES)))

    out = np.zeros((B, L, C), dtype=np.float32)
    for core in range(NCORES):
        b = core // 4
        out[b] += np.asarray(res.results[core]["outp"], dtype=np.float32)
    out += bo[None, None, :]
    return out


# revision 33
# speedup vs baseline: 1.0349x; 1.0349x over previous
"""Trainium2 Bass kernel for nn_CrossAttention (B=2, L=1024, S=2048, DIM=1024, H=16 heads).

Sharding: tensor-parallel over heads x data-parallel over batch.
Core c handles batch b = c//4 and head-group g = c%4 (4 heads = 256 of the
1024 hidden channels).  Each core computes, for its (b, g):

    QT = (Wq_g)^T x_q^T          [256, 1024]   (m on partitions)
    KT = (Wk_g)^T x_k^T          [256, 2048]
    V  = x_v Wv_g                [2048, 256]   (s on partitions)
    per head h (d=64):
        ST_h = KT_h^T' ...       S^T[s, l] = k_s . q_l   (s on partitions)
        P_h  = exp(SCALE * ST_h)            (unnormalized, s on partitions)
        [O^T_h ; sums_h] = [V_h | 1]^T @ P_h   (ones-column folds the softmax
                                                denominator into the matmul)
        XgT_h = O^T_h * (1/sums_h)          (broadcast via K=1 matmul)
    out_partial = XgT^T @ Wo_g   [1024, 1024]  (bf16)

Host gathers: out[b] = sum_g out_partial[4b+g] + bo.

Weights are pre-rearranged on the host so every DMA is contiguous.
Activations and weights stream in bf16; the attention core runs in
fp32r (FP22) with fp32 PSUM accumulation.
"""

import sys

if "/opt/trn_rl_repo" not in sys.path:
    sys.path.insert(0, "/opt/trn_rl_repo")

import numpy as np

B, L, S, C = 2, 1024, 2048, 1024
NH, D = 16, 64          # total heads, head dim
HPC = 4                 # heads per core
M = HPC * D             # 256 output channels per core
SCALE = D ** -0.5
P = 128                 # partitions
NCORES = 8
CK = C // P             # 8 c-tiles
NST = S // P            # 16 s-tiles
LCH = 512               # l-chunk
NLCH = L // LCH         # 2

_cache = {}


def _build():
    import concourse.tile as tile
    from concourse import mybir, bacc

    f32 = mybir.dt.float32
    f32r = mybir.dt.float32r
    bf16 = mybir.dt.bfloat16

    nc = bacc.Bacc("TRN2", target_bir_lowering=False, debug=False)

    # xq/xk pre-packed host-side as [p, ckpair, ck_in_pair, cols] so each
    # per-pair DMA reads 4-8KB contiguous per partition
    xqP = nc.dram_tensor("xqP", [P, CK // 2, 2, L], bf16, kind="ExternalInput")
    xkP = nc.dram_tensor("xkP", [P, CK // 2, 2, S], bf16, kind="ExternalInput")
    xvT = nc.dram_tensor("xvT", [C, S], bf16, kind="ExternalInput")
    # weights pre-rearranged host-side: [p, ck, m] / [p, kt, n]
    wq = nc.dram_tensor("wq", [P, CK, M], bf16, kind="ExternalInput")
    wk = nc.dram_tensor("wk", [P, CK, M], bf16, kind="ExternalInput")
    wv = nc.dram_tensor("wv", [P, CK, M], bf16, kind="ExternalInput")
    wo = nc.dram_tensor("wo", [P, M // P, C], bf16, kind="ExternalInput")
    outp = nc.dram_tensor("outp", [L, C], bf16, kind="ExternalOutput")

    with tile.TileContext(nc) as tc:
        from concourse import library_config
        with tc.tile_pool(name="singles", bufs=1) as singles, \
             tc.tile_pool(name="acts", bufs=4) as acts, \
             tc.tile_pool(name="pts", bufs=36) as pts, \
             tc.tile_pool(name="small", bufs=2) as small, \
             tc.tile_pool(name="obuf", bufs=3) as obuf:

            # ---- persistent SBUF ----
            wq_sb = singles.tile([P, CK, M], bf16, tag="wq")
            wk_sb = singles.tile([P, CK, M], bf16, tag="wk")
            wv_sb = singles.tile([P, CK, M], bf16, tag="wv")
            wo_sb = singles.tile([P, M // P, C], bf16, tag="wo")
            xv_sb = singles.tile([P, CK, S], bf16, tag="xv")
            # A-critical loads split across the sync HWDGE queue and the
            # gpsimd SWDGE queue (both ~210-240 GB/s; together ~HBM cap).
            # Neither touches the ACT queue, which carries exp.
            nc.sync.dma_start(wq_sb[:], wq[:, :, :])

            # per-chunk tiles so dependency tracking stays fine-grained
            ktt = [[singles.tile([P, 512], bf16, tag=f"kt{mt}{nch}",
                                 name=f"ktt{mt}{nch}")
                    for nch in range(4)] for mt in range(2)]
            qtt = [[singles.tile([P, 512], bf16, tag=f"qt{mt}{lh}",
                                 name=f"qtt{mt}{lh}")
                    for lh in range(2)] for mt in range(2)]
            # per-head lhsT [s, 128]: col 0 = ones (sums -> psum row 0),
            # cols 64..127 = V (O^T -> psum rows 64..127, 32-aligned for DVE),
            # cols 1..63 junk -> unread psum rows
            vt = [singles.tile([P, HPC, P], bf16, tag=f"v{st}", name=f"vt{st}")
                  for st in range(NST)]
            xgt_sb = singles.tile([P, 2, L], bf16, tag="xgt")
            for st in range(NST):
                nc.gpsimd.memset(vt[st][:, :, 0:1], 1.0)

            # =========== Phase A: QT + KT projections ===========
            with tc.tile_pool(name="ps_proj", bufs=8, space="PSUM") as psp:
                # --- QT projection ---
                qt_ps = [psp.tile([P, 512], f32, tag="pp", name=f"qtps{i}") for i in range(4)]
                for ck in range(CK):
                    xq_t = acts.tile([P, L], bf16, tag="act")
                    nc.sync.dma_start(xq_t[:], xqP[:, ck // 2, ck % 2, :])
                    for mt in range(2):
                        for lh in range(2):
                            nc.tensor.matmul(
                                qt_ps[mt * 2 + lh][:],
                                wq_sb[:, ck, mt * P:(mt + 1) * P],
                                xq_t[:, lh * 512:(lh + 1) * 512],
                                start=(ck == 0), stop=(ck == CK - 1))
                            if ck == CK - 1:
                                if lh == 0:
                                    nc.scalar.copy(qtt[mt][lh][:],
                                                   qt_ps[mt * 2 + lh][:])
                                else:
                                    nc.vector.tensor_copy(qtt[mt][lh][:],
                                                          qt_ps[mt * 2 + lh][:])

                # --- KT projection ---
                kt_ps = [psp.tile([P, 512], f32, tag="pp", name=f"ktps{i}") for i in range(8)]
                nc.sync.dma_start(wk_sb[:], wk[:, :, :])
                for ck in range(CK):
                    xk_t = acts.tile([P, S], bf16, tag="actk")
                    nc.sync.dma_start(xk_t[:], xkP[:, ck // 2, ck % 2, :])
                    for mt in range(2):
                        for nch in range(4):
                            nc.tensor.matmul(
                                kt_ps[mt * 4 + nch][:],
                                wk_sb[:, ck, mt * P:(mt + 1) * P],
                                xk_t[:, nch * 512:(nch + 1) * 512],
                                start=(ck == 0), stop=(ck == CK - 1))
                            if ck == CK - 1:
                                if nch % 2 == 0:
                                    nc.scalar.copy(ktt[mt][nch][:],
                                                   kt_ps[mt * 4 + nch][:])
                                else:
                                    nc.vector.tensor_copy(ktt[mt][nch][:],
                                                          kt_ps[mt * 4 + nch][:])
                # wv/wo on scalar; xv (half-row-major chunks) trails
                # the A-critical stream on sync
                nc.sync.dma_start(wv_sb[:], wv[:, :, :])
                nc.sync.dma_start(wo_sb[:], wo[:, :, :])
                for h in range(2):
                    for ck in range(CK):
                        nc.sync.dma_start(
                            xv_sb[:, ck, h * 1024:(h + 1) * 1024],
                            xvT[ck * P:(ck + 1) * P, h * 1024:(h + 1) * 1024])
                # gpsimd library (for partition_broadcast): the load blocks
                # the Pool queue ~10us, harmless here (no SWDGE traffic)
                nc.gpsimd.load_library(library_config.standard)

            # =========== Phases B-E: V-proj / attention pipeline ===========
            def st_step(lch, pair, st):
                """ST pair matmuls + exp; returns the PT tile."""
                nch, co = st // 4, (st % 4) * P
                st_ps = pst.tile([P, 2, LCH], f32, tag="st", name=f"stps_{lch}_{pair}_{st}")
                nc.tensor.matmul(
                    st_ps[:, 0, :], ktt[pair][nch][0:D, co:co + P],
                    qtt[pair][lch][0:D, :], start=True, stop=True)
                nc.tensor.matmul(
                    st_ps[:, 1, :], ktt[pair][nch][D:P, co:co + P],
                    qtt[pair][lch][D:P, :], start=True, stop=True,
                    tile_position=(64, 0))
                pt_t = pts.tile([P, 2, LCH], bf16, tag="pt", name=f"pt_{lch}_{pair}_{st}")
                nc.scalar.activation(pt_t[:], st_ps[:],
                                     mybir.ActivationFunctionType.Exp, scale=SCALE)
                return pt_t

            def o_step(o_ps, lch, pair, st, pt_t):
                for hh in range(2):
                    nc.tensor.matmul(
                        o_ps[hh][:], vt[st][:, pair * 2 + hh, :], pt_t[:, hh, :],
                        start=(st == 0), stop=(st == NST - 1))

            def norm_pair(lch, pair, o_ps):
                """Tensor-free softmax normalization: sums sit on partition 0
                (ones column at index 0); approx-recip straight from PSUM,
                gpsimd partition-broadcast, then DVE multiply."""
                lsl = slice(lch * LCH, (lch + 1) * LCH)
                for hh in range(2):
                    rc = small.tile([1, LCH], f32, tag="rc")
                    bc_sb = small.tile([D, LCH], f32, tag="bc")
                    with nc.allow_low_precision(reason="softmax denom approx recip"):
                        nc.vector.reciprocal_approx_fast(rc[0:1, :],
                                                         o_ps[hh][0:1, :])
                    nc.gpsimd.partition_broadcast(bc_sb[:, :], rc[0:1, :])
                    nc.vector.tensor_mul(
                        xgt_sb[hh * D:(hh + 1) * D, pair, lsl],
                        o_ps[hh][P - D:P, :], bc_sb[:])

            ob_tiles = {}
            ob_n = [0]

            def wo_copy_store(lt, nch, wo_ps):
                if lt not in ob_tiles:
                    ob_tiles[lt] = obuf.tile([P, C], bf16, tag="ob", name=f"ob_{lt}")
                ob_sb = ob_tiles[lt]
                # PSUM->SBUF cast on ACT (idle once the exp stream ends);
                # combined per-lt stores alternating sync/gpsimd queues
                nc.scalar.copy(ob_sb[:, nch * 512:(nch + 1) * 512], wo_ps[:])
                ob_n[0] += 1
                if nch == 1:
                    eng = nc.sync if lt % 2 == 0 else nc.gpsimd
                    eng.dma_start(
                        outp[lt * P:(lt + 1) * P, :], ob_tiles.pop(lt)[:])

            def wo_step(ps1, lt, nch):
                wo_ps = ps1.tile([P, 512], f32, tag="ps1", name=f"wops_{lt}_{nch}")
                for kt in range(2):
                    nc.tensor.matmul(
                        wo_ps[:], xgt_sb[:, kt, lt * P:(lt + 1) * P],
                        wo_sb[:, kt, nch * 512:(nch + 1) * 512],
                        start=(kt == 0), stop=(kt == 1))
                wo_copy_store(lt, nch, wo_ps)

            pt0 = {}   # (pair, st) -> PT tile for lch 0
            pt1 = {}
            # PSUM pool lifetimes overlap non-hierarchically; use the two
            # allocator sides as independent stacks:
            #   right: pst [B,C] -> pse [D,E];  left: psv [B] -> ps1 [C,D,E]
            pst = tc.alloc_tile_pool(name="ps_st", bufs=2, side="right", space="PSUM")
            psv = tc.alloc_tile_pool(name="ps_v", bufs=4, side="left", space="PSUM")

            # --- Phase B: V projection (4 quarters) interleaved with
            #     lch0's ST+exp steps; first 4 ST steps up front so exp
            #     starts as soon as KT lands (xv may still be in flight) ---
            step = 0
            for _ in range(4):
                pair, st = divmod(step, NST)
                pt0[(pair, st)] = st_step(0, pair, st)
                step += 1
            for q in range(4):
                v_ps = [psv.tile([P, M], f32, tag="vp", name=f"vps{q}_{i}")
                        for i in range(4)]
                for ck in range(CK):
                    for st4 in range(4):
                        nc.tensor.matmul(
                            v_ps[st4][:],
                            xv_sb[:, ck, q * 4 * P + st4 * P:
                                  q * 4 * P + (st4 + 1) * P],
                            wv_sb[:, ck, :],
                            start=(ck == 0), stop=(ck == CK - 1))
                    if step < 2 * NST:
                        pair, st = divmod(step, NST)
                        pt0[(pair, st)] = st_step(0, pair, st)
                        step += 1
                for st4 in range(4):
                    st = q * 4 + st4
                    nc.vector.tensor_copy(
                        vt[st][:, :, P - D:P],
                        v_ps[st4][:].rearrange("p (h d) -> p h d", h=HPC))

            psv.release()
            ps1 = tc.alloc_tile_pool(name="ps1", bufs=4, side="left", space="PSUM")

            # --- Phase C0: O(0,p0) + ST/exp(1,p0) ---
            o00 = [ps1.tile([P, LCH], f32, tag="ps1",
                            name=f"ops0_0_{i}") for i in range(2)]
            for st in range(NST):
                o_step(o00, 0, 0, st, pt0.pop((0, st)))
                pt1[(0, st)] = st_step(1, 0, st)
            norm_pair(0, 0, o00)

            # --- Phase C1: O(0,p1) + ST/exp(1,p1) + O(1,p0) ---
            o01 = [ps1.tile([P, LCH], f32, tag="ps1",
                            name=f"ops0_1_{i}") for i in range(2)]
            o10 = [ps1.tile([P, LCH], f32, tag="ps1",
                            name=f"ops1_0_{i}") for i in range(2)]
            for st in range(NST):
                o_step(o01, 0, 1, st, pt0.pop((1, st)))
                pt1[(1, st)] = st_step(1, 1, st)
                o_step(o10, 1, 0, st, pt1.pop((0, st)))
            norm_pair(0, 1, o01)
            norm_pair(1, 0, o10)

            pst.release()
            pse = tc.alloc_tile_pool(name="ps_e", bufs=4, side="right", space="PSUM")

            # --- Phase D: O(1,p1) + Wo(lch0) + early kt0 of Wo(lch1) ---
            wo_jobs0 = [(lt, nch) for lt in range(4) for nch in range(2)]
            eps = {}
            o11 = [ps1.tile([P, LCH], f32, tag="ps1",
                            name=f"ops1_1_{i}") for i in range(2)]
            for st in range(NST):
                o_step(o11, 1, 1, st, pt1.pop((1, st)))
                if st % 2 == 1 and wo_jobs0:
                    wo_step(ps1, *wo_jobs0.pop(0))
                if st in (5, 7, 9, 11):
                    # early kt=0 half of phase-E Wo (xgt lch1 pair0 ready)
                    lt = 4 + (st - 5) // 2
                    wo_ps = pse.tile([P, 512], f32, tag="pse",
                                     name=f"ewo_{lt}")
                    nc.tensor.matmul(
                        wo_ps[:], xgt_sb[:, 0, lt * P:(lt + 1) * P],
                        wo_sb[:, 0, 0:512], start=True, stop=False)
                    eps[lt] = wo_ps
            norm_pair(1, 1, o11)
            for lt, nch in wo_jobs0:
                wo_step(ps1, lt, nch)

            # --- Phase E: Wo(lch1) ---
            for lt in range(4, 8):
                # finish the early-started nch=0 job
                wo_ps = eps.pop(lt)
                nc.tensor.matmul(
                    wo_ps[:], xgt_sb[:, 1, lt * P:(lt + 1) * P],
                    wo_sb[:, 1, 0:512], start=False, stop=True)
                wo_copy_store(lt, 0, wo_ps)
                wo_step(ps1, lt, 1)

            pse.release()
            ps1.release()

    nc.compile()
    return nc


def _get_nc():
    if "nc" not in _cache:
        _cache["nc"] = _build()
    return _cache["nc"]


def _make_in_maps(inputs):
    import ml_dtypes

    bf16 = ml_dtypes.bfloat16
    query = np.asarray(inputs["query"], dtype=np.float32)
    key = np.asarray(inputs["key"], dtype=np.float32)
    value = np.asarray(inputs["value"], dtype=np.float32)
    Wq = np.asarray(inputs["Wq"], dtype=np.float32)
    Wk = np.asarray(inputs["Wk"], dtype=np.float32)
    Wv = np.asarray(inputs["Wv"], dtype=np.float32)
    Wo = np.asarray(inputs["Wo"], dtype=np.float32)

    def wsplit(W, g):
        # [C, M] slice -> [P, CK, M] so the DMA is fully contiguous
        return np.ascontiguousarray(
            W[:, g * M:(g + 1) * M].reshape(CK, P, M).transpose(1, 0, 2)
        ).astype(bf16)

    def wosplit(W, g):
        # [M, C] slice -> [P, M//P, C]
        return np.ascontiguousarray(
            W[g * M:(g + 1) * M, :].reshape(M // P, P, C).transpose(1, 0, 2)
        ).astype(bf16)

    def pack_pairs(xT):
        # [C, cols] -> [P, CK//2, 2, cols]
        cols = xT.shape[1]
        return np.ascontiguousarray(
            xT.reshape(CK // 2, 2, P, cols).transpose(2, 0, 1, 3)).astype(bf16)

    qP = [pack_pairs(query[b].T) for b in range(B)]
    kP = [pack_pairs(key[b].T) for b in range(B)]
    vT = [np.ascontiguousarray(value[b].T).astype(bf16) for b in range(B)]
    wq_s = [wsplit(Wq, g) for g in range(4)]
    wk_s = [wsplit(Wk, g) for g in range(4)]
    wv_s = [wsplit(Wv, g) for g in range(4)]
    wo_s = [wosplit(Wo, g) for g in range(4)]

    in_maps = []
    for core in range(NCORES):
        b, g = core // 4, core % 4
        in_maps.append({
            "xqP": qP[b], "xkP": kP[b], "xvT": vT[b],
            "wq": wq_s[g], "wk": wk_s[g], "wv": wv_s[g], "wo": wo_s[g],
        })
    return in_maps


def kernel(query, key, value, Wq, Wk, Wv, Wo, bo):
    from concourse.bass_utils import run_bass_kernel_spmd

    nc = _get_nc()
    bo = np.asarray(bo, dtype=np.float32)
    in_maps = _make_in_maps(dict(query=query, key=key, value=value,
                                 Wq=Wq, Wk=Wk, Wv=Wv, Wo=Wo))

    res = run_bass_kernel_spmd(nc, in_maps, core_ids=list(range(NCORES)))

    out = np.zeros((B, L, C), dtype=np.float32)
    for core in range(NCORES):
        b = core // 4
        out[b] += np.asarray(res.results[core]["outp"], dtype=np.float32)
    out += bo[None, None, :]
    return out


# revision 34
# speedup vs baseline: 1.0391x; 1.0041x over previous
"""Trainium2 Bass kernel for nn_CrossAttention (B=2, L=1024, S=2048, DIM=1024, H=16 heads).

Sharding: tensor-parallel over heads x data-parallel over batch.
Core c handles batch b = c//4 and head-group g = c%4 (4 heads = 256 of the
1024 hidden channels).  Each core computes, for its (b, g):

    QT = (Wq_g)^T x_q^T          [256, 1024]   (m on partitions)
    KT = (Wk_g)^T x_k^T          [256, 2048]
    V  = x_v Wv_g                [2048, 256]   (s on partitions)
    per head h (d=64):
        ST_h = KT_h^T' ...       S^T[s, l] = k_s . q_l   (s on partitions)
        P_h  = exp(SCALE * ST_h)            (unnormalized, s on partitions)
        [O^T_h ; sums_h] = [V_h | 1]^T @ P_h   (ones-column folds the softmax
                                                denominator into the matmul)
        XgT_h = O^T_h * (1/sums_h)          (broadcast via K=1 matmul)
    out_partial = XgT^T @ Wo_g   [1024, 1024]  (bf16)

Host gathers: out[b] = sum_g out_partial[4b+g] + bo.

Weights are pre-rearranged on the host so every DMA is contiguous.
Activations and weights stream in bf16; the attention core runs in
fp32r (FP22) with fp32 PSUM accumulation.
"""

import sys

if "/opt/trn_rl_repo" not in sys.path:
    sys.path.insert(0, "/opt/trn_rl_repo")

import numpy as np

B, L, S, C = 2, 1024, 2048, 1024
NH, D = 16, 64          # total heads, head dim
HPC = 4                 # heads per core
M = HPC * D             # 256 output channels per core
SCALE = D ** -0.5
P = 128                 # partitions
NCORES = 8
CK = C // P             # 8 c-tiles
NST = S // P            # 16 s-tiles
LCH = 512               # l-chunk
NLCH = L // LCH         # 2

_cache = {}


def _build():
    import concourse.tile as tile
    from concourse import mybir, bacc

    f32 = mybir.dt.float32
    f32r = mybir.dt.float32r
    bf16 = mybir.dt.bfloat16

    nc = bacc.Bacc("TRN2", target_bir_lowering=False, debug=False)

    # xq/xk pre-packed host-side as [p, ckpair, ck_in_pair, cols] so each
    # per-pair DMA reads 4-8KB contiguous per partition
    xqP = nc.dram_tensor("xqP", [P, CK // 2, 2, L], bf16, kind="ExternalInput")
    xkP = nc.dram_tensor("xkP", [P, CK // 2, 2, S], bf16, kind="ExternalInput")
    xvT = nc.dram_tensor("xvT", [C, S], bf16, kind="ExternalInput")
    # weights pre-rearranged host-side: [p, ck, m] / [p, kt, n]
    wq = nc.dram_tensor("wq", [P, CK, M], bf16, kind="ExternalInput")
    wk = nc.dram_tensor("wk", [P, CK, M], bf16, kind="ExternalInput")
    wv = nc.dram_tensor("wv", [P, CK, M], bf16, kind="ExternalInput")
    wo = nc.dram_tensor("wo", [P, M // P, C], bf16, kind="ExternalInput")
    outp = nc.dram_tensor("outp", [L, C], bf16, kind="ExternalOutput")

    with tile.TileContext(nc) as tc:
        from concourse import library_config
        with tc.tile_pool(name="singles", bufs=1) as singles, \
             tc.tile_pool(name="acts", bufs=4) as acts, \
             tc.tile_pool(name="pts", bufs=36) as pts, \
             tc.tile_pool(name="small", bufs=2) as small, \
             tc.tile_pool(name="obuf", bufs=3) as obuf:

            # ---- persistent SBUF ----
            wq_sb = singles.tile([P, CK, M], bf16, tag="wq")
            wk_sb = singles.tile([P, CK, M], bf16, tag="wk")
            wv_sb = singles.tile([P, CK, M], bf16, tag="wv")
            wo_sb = singles.tile([P, M // P, C], bf16, tag="wo")
            xv_sb = singles.tile([P, CK, S], bf16, tag="xv")
            # A-critical loads split across the sync HWDGE queue and the
            # gpsimd SWDGE queue (both ~210-240 GB/s; together ~HBM cap).
            # Neither touches the ACT queue, which carries exp.
            nc.sync.dma_start(wq_sb[:], wq[:, :, :])

            # per-chunk tiles so dependency tracking stays fine-grained
            ktt = [[singles.tile([P, 512], bf16, tag=f"kt{mt}{nch}",
                                 name=f"ktt{mt}{nch}")
                    for nch in range(4)] for mt in range(2)]
            qtt = [[singles.tile([P, 512], bf16, tag=f"qt{mt}{lh}",
                                 name=f"qtt{mt}{lh}")
                    for lh in range(2)] for mt in range(2)]
            # per-head lhsT [s, 128]: col 0 = ones (sums -> psum row 0),
            # cols 64..127 = V (O^T -> psum rows 64..127, 32-aligned for DVE),
            # cols 1..63 junk -> unread psum rows
            vt = [singles.tile([P, HPC, P], bf16, tag=f"v{st}", name=f"vt{st}")
                  for st in range(NST)]
            xgt_sb = singles.tile([P, 2, L], bf16, tag="xgt")
            for st in range(NST):
                nc.gpsimd.memset(vt[st][:, :, 0:1], 1.0)

            # =========== Phase A: QT + KT projections ===========
            with tc.tile_pool(name="ps_proj", bufs=8, space="PSUM") as psp:
                # --- QT projection ---
                qt_ps = [psp.tile([P, 512], f32, tag="pp", name=f"qtps{i}") for i in range(4)]
                xq_ts = {}
                for j in range(CK // 2):
                    xq_ts[j] = acts.tile([P, 2, L], bf16, tag="act", name=f"xq{j}")
                    nc.sync.dma_start(xq_ts[j][:], xqP[:, j, :, :])
                for ck in range(CK):
                    xq_t = xq_ts[ck // 2]
                    for mt in range(2):
                        for lh in range(2):
                            nc.tensor.matmul(
                                qt_ps[mt * 2 + lh][:],
                                wq_sb[:, ck, mt * P:(mt + 1) * P],
                                xq_t[:, ck % 2, lh * 512:(lh + 1) * 512],
                                start=(ck == 0), stop=(ck == CK - 1))
                            if ck == CK - 1:
                                if lh == 0:
                                    nc.scalar.copy(qtt[mt][lh][:],
                                                   qt_ps[mt * 2 + lh][:])
                                else:
                                    nc.vector.tensor_copy(qtt[mt][lh][:],
                                                          qt_ps[mt * 2 + lh][:])

                # --- KT projection ---
                kt_ps = [psp.tile([P, 512], f32, tag="pp", name=f"ktps{i}") for i in range(8)]
                nc.sync.dma_start(wk_sb[:], wk[:, :, :])
                for ck in range(CK):
                    xk_t = acts.tile([P, S], bf16, tag="actk")
                    nc.sync.dma_start(xk_t[:], xkP[:, ck // 2, ck % 2, :])
                    for mt in range(2):
                        for nch in range(4):
                            nc.tensor.matmul(
                                kt_ps[mt * 4 + nch][:],
                                wk_sb[:, ck, mt * P:(mt + 1) * P],
                                xk_t[:, nch * 512:(nch + 1) * 512],
                                start=(ck == 0), stop=(ck == CK - 1))
                            if ck == CK - 1:
                                if nch % 2 == 0:
                                    nc.scalar.copy(ktt[mt][nch][:],
                                                   kt_ps[mt * 4 + nch][:])
                                else:
                                    nc.vector.tensor_copy(ktt[mt][nch][:],
                                                          kt_ps[mt * 4 + nch][:])
                # wv/wo on scalar; xv (half-row-major chunks) trails
                # the A-critical stream on sync
                nc.sync.dma_start(wv_sb[:], wv[:, :, :])
                nc.sync.dma_start(wo_sb[:], wo[:, :, :])
                for h in range(2):
                    for ck in range(CK):
                        nc.sync.dma_start(
                            xv_sb[:, ck, h * 1024:(h + 1) * 1024],
                            xvT[ck * P:(ck + 1) * P, h * 1024:(h + 1) * 1024])
                # gpsimd library (for partition_broadcast): the load blocks
                # the Pool queue ~10us, harmless here (no SWDGE traffic)
                nc.gpsimd.load_library(library_config.standard)

            # =========== Phases B-E: V-proj / attention pipeline ===========
            def st_step(lch, pair, st):
                """ST pair matmuls + exp; returns the PT tile."""
                nch, co = st // 4, (st % 4) * P
                st_ps = pst.tile([P, 2, LCH], f32, tag="st", name=f"stps_{lch}_{pair}_{st}")
                nc.tensor.matmul(
                    st_ps[:, 0, :], ktt[pair][nch][0:D, co:co + P],
                    qtt[pair][lch][0:D, :], start=True, stop=True)
                nc.tensor.matmul(
                    st_ps[:, 1, :], ktt[pair][nch][D:P, co:co + P],
                    qtt[pair][lch][D:P, :], start=True, stop=True,
                    tile_position=(64, 0))
                pt_t = pts.tile([P, 2, LCH], bf16, tag="pt", name=f"pt_{lch}_{pair}_{st}")
                nc.scalar.activation(pt_t[:], st_ps[:],
                                     mybir.ActivationFunctionType.Exp, scale=SCALE)
                return pt_t

            def o_step(o_ps, lch, pair, st, pt_t):
                for hh in range(2):
                    nc.tensor.matmul(
                        o_ps[hh][:], vt[st][:, pair * 2 + hh, :], pt_t[:, hh, :],
                        start=(st == 0), stop=(st == NST - 1))

            def norm_pair(lch, pair, o_ps):
                """Tensor-free softmax normalization: sums sit on partition 0
                (ones column at index 0); approx-recip straight from PSUM,
                gpsimd partition-broadcast, then DVE multiply."""
                lsl = slice(lch * LCH, (lch + 1) * LCH)
                for hh in range(2):
                    rc = small.tile([1, LCH], f32, tag="rc")
                    bc_sb = small.tile([D, LCH], f32, tag="bc")
                    with nc.allow_low_precision(reason="softmax denom approx recip"):
                        nc.vector.reciprocal_approx_fast(rc[0:1, :],
                                                         o_ps[hh][0:1, :])
                    nc.gpsimd.partition_broadcast(bc_sb[:, :], rc[0:1, :])
                    nc.vector.tensor_mul(
                        xgt_sb[hh * D:(hh + 1) * D, pair, lsl],
                        o_ps[hh][P - D:P, :], bc_sb[:])

            ob_tiles = {}
            ob_n = [0]

            def wo_copy_store(lt, nch, wo_ps):
                if lt not in ob_tiles:
                    ob_tiles[lt] = obuf.tile([P, C], bf16, tag="ob", name=f"ob_{lt}")
                ob_sb = ob_tiles[lt]
                # PSUM->SBUF cast on ACT (idle once the exp stream ends);
                # combined per-lt stores alternating sync/gpsimd queues
                if ob_n[0] % 2 == 0:
                    nc.scalar.copy(ob_sb[:, nch * 512:(nch + 1) * 512], wo_ps[:])
                else:
                    nc.vector.tensor_copy(
                        ob_sb[:, nch * 512:(nch + 1) * 512], wo_ps[:])
                ob_n[0] += 1
                if nch == 1:
                    eng = nc.sync if lt % 2 == 0 else nc.gpsimd
                    eng.dma_start(
                        outp[lt * P:(lt + 1) * P, :], ob_tiles.pop(lt)[:])

            def wo_step(ps1, lt, nch):
                wo_ps = ps1.tile([P, 512], f32, tag="ps1", name=f"wops_{lt}_{nch}")
                for kt in range(2):
                    nc.tensor.matmul(
                        wo_ps[:], xgt_sb[:, kt, lt * P:(lt + 1) * P],
                        wo_sb[:, kt, nch * 512:(nch + 1) * 512],
                        start=(kt == 0), stop=(kt == 1))
                wo_copy_store(lt, nch, wo_ps)

            pt0 = {}   # (pair, st) -> PT tile for lch 0
            pt1 = {}
            # PSUM pool lifetimes overlap non-hierarchically; use the two
            # allocator sides as independent stacks:
            #   right: pst [B,C] -> pse [D,E];  left: psv [B] -> ps1 [C,D,E]
            pst = tc.alloc_tile_pool(name="ps_st", bufs=2, side="right", space="PSUM")
            psv = tc.alloc_tile_pool(name="ps_v", bufs=4, side="left", space="PSUM")

            # --- Phase B: V projection (4 quarters) interleaved with
            #     lch0's ST+exp steps; first 4 ST steps up front so exp
            #     starts as soon as KT lands (xv may still be in flight) ---
            step = 0
            for _ in range(4):
                pair, st = divmod(step, NST)
                pt0[(pair, st)] = st_step(0, pair, st)
                step += 1
            for q in range(4):
                v_ps = [psv.tile([P, M], f32, tag="vp", name=f"vps{q}_{i}")
                        for i in range(4)]
                for ck in range(CK):
                    for st4 in range(4):
                        nc.tensor.matmul(
                            v_ps[st4][:],
                            xv_sb[:, ck, q * 4 * P + st4 * P:
                                  q * 4 * P + (st4 + 1) * P],
                            wv_sb[:, ck, :],
                            start=(ck == 0), stop=(ck == CK - 1))
                    if step < 2 * NST:
                        pair, st = divmod(step, NST)
                        pt0[(pair, st)] = st_step(0, pair, st)
                        step += 1
                for st4 in range(4):
                    st = q * 4 + st4
                    nc.vector.tensor_copy(
                        vt[st][:, :, P - D:P],
                        v_ps[st4][:].rearrange("p (h d) -> p h d", h=HPC))

            psv.release()
            ps1 = tc.alloc_tile_pool(name="ps1", bufs=4, side="left", space="PSUM")

            # --- Phase C0: O(0,p0) + ST/exp(1,p0) ---
            o00 = [ps1.tile([P, LCH], f32, tag="ps1",
                            name=f"ops0_0_{i}") for i in range(2)]
            for st in range(NST):
                o_step(o00, 0, 0, st, pt0.pop((0, st)))
                pt1[(0, st)] = st_step(1, 0, st)
            norm_pair(0, 0, o00)

            # --- Phase C1: O(0,p1) + ST/exp(1,p1) + O(1,p0) ---
            o01 = [ps1.tile([P, LCH], f32, tag="ps1",
                            name=f"ops0_1_{i}") for i in range(2)]
            o10 = [ps1.tile([P, LCH], f32, tag="ps1",
                            name=f"ops1_0_{i}") for i in range(2)]
            for st in range(NST):
                o_step(o01, 0, 1, st, pt0.pop((1, st)))
                pt1[(1, st)] = st_step(1, 1, st)
                o_step(o10, 1, 0, st, pt1.pop((0, st)))
            norm_pair(0, 1, o01)
            norm_pair(1, 0, o10)

            pst.release()
            pse = tc.alloc_tile_pool(name="ps_e", bufs=4, side="right", space="PSUM")

            # --- Phase D: O(1,p1) + Wo(lch0) + early kt0 of Wo(lch1) ---
            wo_jobs0 = [(lt, nch) for lt in range(4) for nch in range(2)]
            eps = {}
            o11 = [ps1.tile([P, LCH], f32, tag="ps1",
                            name=f"ops1_1_{i}") for i in range(2)]
            for st in range(NST):
                o_step(o11, 1, 1, st, pt1.pop((1, st)))
                if st % 2 == 1 and wo_jobs0:
                    wo_step(ps1, *wo_jobs0.pop(0))
                if st in (5, 7, 9, 11):
                    # early kt=0 half of phase-E Wo (xgt lch1 pair0 ready)
                    lt = 4 + (st - 5) // 2
                    wo_ps = pse.tile([P, 512], f32, tag="pse",
                                     name=f"ewo_{lt}")
                    nc.tensor.matmul(
                        wo_ps[:], xgt_sb[:, 0, lt * P:(lt + 1) * P],
                        wo_sb[:, 0, 0:512], start=True, stop=False)
                    eps[lt] = wo_ps
            norm_pair(1, 1, o11)
            for lt, nch in wo_jobs0:
                wo_step(ps1, lt, nch)

            # --- Phase E: Wo(lch1) ---
            for lt in range(4, 8):
                # finish the early-started nch=0 job
                wo_ps = eps.pop(lt)
                nc.tensor.matmul(
                    wo_ps[:], xgt_sb[:, 1, lt * P:(lt + 1) * P],
                    wo_sb[:, 1, 0:512], start=False, stop=True)
                wo_copy_store(lt, 0, wo_ps)
                wo_step(ps1, lt, 1)

            pse.release()
            ps1.release()

    nc.compile()
    return nc


def _get_nc():
    if "nc" not in _cache:
        _cache["nc"] = _build()
    return _cache["nc"]


def _make_in_maps(inputs):
    import ml_dtypes

    bf16 = ml_dtypes.bfloat16
    query = np.asarray(inputs["query"], dtype=np.float32)
    key = np.asarray(inputs["key"], dtype=np.float32)
    value = np.asarray(inputs["value"], dtype=np.float32)
    Wq = np.asarray(inputs["Wq"], dtype=np.float32)
    Wk = np.asarray(inputs["Wk"], dtype=np.float32)
    Wv = np.asarray(inputs["Wv"], dtype=np.float32)
    Wo = np.asarray(inputs["Wo"], dtype=np.float32)

    def wsplit(W, g):
        # [C, M] slice -> [P, CK, M] so the DMA is fully contiguous
        return np.ascontiguousarray(
            W[:, g * M:(g + 1) * M].reshape(CK, P, M).transpose(1, 0, 2)
        ).astype(bf16)

    def wosplit(W, g):
        # [M, C] slice -> [P, M//P, C]
        return np.ascontiguousarray(
            W[g * M:(g + 1) * M, :].reshape(M // P, P, C).transpose(1, 0, 2)
        ).astype(bf16)

    def pack_pairs(xT):
        # [C, cols] -> [P, CK//2, 2, cols]
        cols = xT.shape[1]
        return np.ascontiguousarray(
            xT.reshape(CK // 2, 2, P, cols).transpose(2, 0, 1, 3)).astype(bf16)

    qP = [pack_pairs(query[b].T) for b in range(B)]
    kP = [pack_pairs(key[b].T) for b in range(B)]
    vT = [np.ascontiguousarray(value[b].T).astype(bf16) for b in range(B)]
    wq_s = [wsplit(Wq, g) for g in range(4)]
    wk_s = [wsplit(Wk, g) for g in range(4)]
    wv_s = [wsplit(Wv, g) for g in range(4)]
    wo_s = [wosplit(Wo, g) for g in range(4)]

    in_maps = []
    for core in range(NCORES):
        b, g = core // 4, core % 4
        in_maps.append({
            "xqP": qP[b], "xkP": kP[b], "xvT": vT[b],
            "wq": wq_s[g], "wk": wk_s[g], "wv": wv_s[g], "wo": wo_s[g],
        })
    return in_maps


def kernel(query, key, value, Wq, Wk, Wv, Wo, bo):
    from concourse.bass_utils import run_bass_kernel_spmd

    nc = _get_nc()
    bo = np.asarray(bo, dtype=np.float32)
    in_maps = _make_in_maps(dict(query=query, key=key, value=value,
                                 Wq=Wq, Wk=Wk, Wv=Wv, Wo=Wo))

    res = run_bass_kernel_spmd(nc, in_maps, core_ids=list(range(NCORES)))

    out = np.zeros((B, L, C), dtype=np.float32)
    for core in range(NCORES):
        b = core // 4
        out[b] += np.asarray(res.results[core]["outp"], dtype=np.float32)
    out += bo[None, None, :]
    return out
